# revision 14
# baseline (speedup 1.0000x reference)
"""Trainium2 Bass kernel: DagnabbitAutoEncoder sequential DAG sweep.

Strategy (8 NeuronCores, SPMD single program, per-core data):
  - Host computes topological levels and a deadline-forced, eager-filled
    pass schedule: nodes are batched into per-(stage, encoder-type) passes;
    28 stages (= DAG depth), one AllGather per stage to exchange computed
    embeddings (fp16) through a shared DRAM buffer `bufH`.
  - Trunk encoder types are partitioned 4-per-core; the shared output-node
    encoder is replicated. Per pass, the core dma_gathers that type's
    weight blob (W1 + repacked W2, fp16 rows) from its private DRAM stack
    into an SBUF slot buffer — fully prefetchable.
  - Parent embeddings are fetched with dma_gather(transpose=True), which
    lands gathered bufH rows as columns: exactly the X^T [512, G] layout
    the tensor engine needs (matmul contracts over the partition dim).
  - Stage 1 GEMM: W1 tiles stationary, X^T moving -> H^T in PSUM;
    ScalarE applies bias+exact-GELU and casts to fp16.
    Stage 2 GEMM: H^T stationary, W2 tiles moving -> Y [g,256] in PSUM;
    bias2 via a ones-row K=1 matmul; VectorE/ScalarE normalize rows to
    sqrt(D); results stored fp32 (final output) + fp16 (exchange).
  - All cores run an identical instruction stream; per-core differences
    (types, node ids, gather indices, weights) are carried in input data,
    padded so shapes/counts match across cores.
"""

import math
import sys

import numpy as np

if "/opt/trn_rl_repo" not in sys.path:
    sys.path.insert(0, "/opt/trn_rl_repo")

NCORES = 8
GCAP = 256  # max nodes per pass slot
TYPES_PER_CORE = 4


# --------------------------------------------------------------------------
# Host-side schedule
# --------------------------------------------------------------------------

class Plan:
    pass


def _build_plan(node_inputs, node_types, num_roots, num_trunk, num_out):
    N = node_inputs.shape[0]
    out_start = num_trunk + num_roots
    is_out = node_types >= out_start
    enc = np.where(is_out, num_trunk, np.clip(node_types, 0, num_trunk - 1))

    # ASAP levels
    level = np.zeros(N, np.int64)
    ni = node_inputs
    for n in range(num_roots, N):
        i0, i1 = ni[n]
        level[n] = (level[i0] + 1) if is_out[n] else max(level[i0], level[i1]) + 1
    S = int(level.max())

    # ALAP deadlines
    alap = np.full(N, S, np.int64)
    for n in range(N - 1, num_roots - 1, -1):
        i0, i1 = ni[n]
        a = alap[n] - 1
        if alap[i0] > a:
            alap[i0] = a
        if (not is_out[n]) and alap[i1] > a:
            alap[i1] = a

    # Greedy deadline-forced scheduling with per-core eager fill.
    # Core for trunk type t is t // TYPES_PER_CORE; output enc (num_trunk) is
    # replicated so its nodes can go anywhere.
    scheduled = np.zeros(N, bool)
    scheduled[:num_roots] = True
    remaining = list(range(num_roots, N))

    stages = []  # list of per-stage dicts
    for s in range(1, S + 1):
        elig = [
            n
            for n in remaining
            if scheduled[ni[n][0]] and (is_out[n] or scheduled[ni[n][1]])
        ]
        by_enc = {}
        for n in elig:
            by_enc.setdefault(int(enc[n]), []).append(n)
        forced = {
            t for t, nodes in by_enc.items() if any(alap[n] == s for n in nodes)
        }
        if not forced:
            continue

        # per-core pass lists of (local_type_idx, [nodes])
        core_passes = [[] for _ in range(NCORES)]

        def add_type(t):
            nodes = by_enc.pop(t)
            if t == num_trunk:
                return nodes  # handled by caller (splittable)
            c = t // TYPES_PER_CORE
            lti = t % TYPES_PER_CORE
            for a in range(0, len(nodes), GCAP):
                core_passes[c].append((lti, nodes[a : a + GCAP]))
            return None

        out_pool = []
        for t in sorted(forced):
            r = add_type(t)
            if r is not None:
                out_pool = r

        K = max(len(p) for p in core_passes)
        K = max(K, 1)

        # place forced output nodes: cores with slack first
        if out_pool:
            chunks = [out_pool[a : a + GCAP] for a in range(0, len(out_pool), GCAP)]
            for ch in chunks:
                c = min(range(NCORES), key=lambda c: len(core_passes[c]))
                core_passes[c].append((TYPES_PER_CORE, ch))
            K = max(K, max(len(p) for p in core_passes))

        # eager fill: cores with fewer passes than K pick up their own
        # eligible (unforced) types, largest first
        for c in range(NCORES):
            while len(core_passes[c]) < K:
                own = [
                    t
                    for t in by_enc
                    if t != num_trunk and t // TYPES_PER_CORE == c
                ]
                if not own:
                    break
                t = max(own, key=lambda t: len(by_enc[t]))
                lti = t % TYPES_PER_CORE
                nodes = by_enc.pop(t)
                for a in range(0, len(nodes), GCAP):
                    core_passes[c].append((lti, nodes[a : a + GCAP]))
            core_passes[c] = core_passes[c][: max(K, len(core_passes[c]))]
        K = max(len(p) for p in core_passes)

        # sort each core's passes by size desc, pad with dummy passes
        for c in range(NCORES):
            core_passes[c].sort(key=lambda p: -len(p[1]))
            while len(core_passes[c]) < K:
                core_passes[c].append((0, []))

        Gs = [
            max(len(core_passes[c][k][1]) for c in range(NCORES))
            for k in range(K)
        ]
        Gs = [max(g, 1) for g in Gs]

        newly = []
        for c in range(NCORES):
            for _, nodes in core_passes[c]:
                newly.extend(nodes)
        for n in newly:
            scheduled[n] = True
        newset = set(newly)
        remaining = [n for n in remaining if n not in newset]

        stages.append(dict(s=s, core_passes=core_passes, Gs=Gs))

    assert not remaining, f"{len(remaining)} nodes unscheduled"

    # ---- layout: bufH rows, cc offsets, output rows, global slot ids ----
    plan = Plan()
    plan.S = len(stages)
    plan.stages = stages
    slot_id = 0
    bufh_off = 0  # offset after the 128 static rows
    out_off = 0
    row_of_node = np.full(N, -1, np.int64)  # bufH row
    outpos_of_node = {}  # node -> (core, oout row)
    for st in stages:
        Gs = st["Gs"]
        K = len(Gs)
        st["slot_ids"] = list(range(slot_id, slot_id + K))
        slot_id += K
        R = sum(Gs)
        st["R"] = R
        st["off"] = bufh_off
        st["out_off"] = out_off
        pre = np.concatenate([[0], np.cumsum(Gs)]).astype(int)
        st["pre"] = pre
        NX = 2 * R
        NX = ((NX + 127) // 128) * 128
        st["NX"] = NX
        for c in range(NCORES):
            for k, (lti, nodes) in enumerate(st["core_passes"][c]):
                for i, n in enumerate(nodes):
                    row_of_node[n] = 128 + bufh_off + c * R + pre[k] + i
                    outpos_of_node[n] = (c, out_off + pre[k] + i)
        bufh_off += NCORES * R
        out_off += R
    plan.bufH_rows = 128 + bufh_off
    plan.R_tot = out_off
    plan.slots_tot = slot_id
    plan.Rmax = max(st["R"] for st in stages)
    plan.NXmax = max(st["NX"] for st in stages)
    plan.Gmax = max(max(st["Gs"]) for st in stages)
    assert plan.bufH_rows < 32768, plan.bufH_rows
    plan.row_of_node = row_of_node
    plan.outpos_of_node = outpos_of_node
    plan.enc = enc
    plan.is_out = is_out
    plan.N = N
    plan.num_roots = num_roots
    plan.num_trunk = num_trunk
    plan.num_out = num_out
    plan.out_start = out_start
    plan.node_inputs = node_inputs
    plan.node_types = node_types
    return plan


def _wrap_idxs(idx_list, num_idxs):
    """int16 index layout for dma_gather: [128, num_idxs//16], index i at
    partition i%16, column i//16, replicated across the 8 Q7 16-partition
    groups."""
    a = np.zeros(num_idxs, np.int16)
    a[: len(idx_list)] = np.asarray(idx_list, np.int16)
    a = a.reshape(num_idxs // 16, 16).T  # [16, cols]
    return np.tile(a, (8, 1))  # [128, cols]


def _build_core_inputs(plan, core, W1, b1, W2, b2, root_emb, output_slot_emb):
    """Per-core input arrays (shapes identical across cores)."""
    num_trunk = plan.num_trunk
    D = root_emb.shape[1]
    H = W1.shape[2]
    assert D == 256 and H == 1024 and W1.shape[1] == 2 * D
    ni_types = [core * TYPES_PER_CORE + j for j in range(TYPES_PER_CORE)] + [num_trunk]

    # weight blob: per local type, 768 rows of 1024 fp16
    # rows 0..511   = W1[t]  (512 x 1024)
    # rows 512..767 = repacked W2[t]: blob[512 + q*128 + p] =
    #                 concat_j W2[t][(4q+j)*128 + p, :]  (j = 0..3)
    rows_per = 3 * D  # 768
    blob = np.zeros((5 * rows_per, H), np.float16)
    for li, t in enumerate(ni_types):
        w1 = W1[t].astype(np.float16)  # [512, 1024]
        blob[li * rows_per : li * rows_per + 2 * D] = w1
        w2 = W2[t].astype(np.float16).reshape(2, 4, 128, D)
        w2 = w2.transpose(0, 2, 1, 3).reshape(2 * 128, 4 * D)  # [256, 1024]
        blob[li * rows_per + 2 * D : (li + 1) * rows_per] = w2

    # per-slot tables
    widx_cols = []
    xidx_cols = []
    bias1 = np.zeros((128, plan.slots_tot * 8), np.float32)
    bias2 = np.zeros((1, plan.slots_tot * D), np.float16)
    nH = H // 128  # number of 128-row b1 tiles (8)
    for st in plan.stages:
        xlist = []
        for k, (lti, nodes) in enumerate(st["core_passes"][core]):
            G = st["Gs"][k]
            sl = st["slot_ids"][k]
            widx_cols.append(_wrap_idxs(lti * rows_per + np.arange(rows_per), rows_per))
            t = ni_types[lti]
            bias1[:, sl * nH : (sl + 1) * nH] = (
                b1[t].astype(np.float32).reshape(nH, 128).T
            )
            bias2[0, sl * D : (sl + 1) * D] = b2[t].astype(np.float16)
            e0 = []
            e1 = []
            for n in nodes:
                i0, i1 = plan.node_inputs[n]
                e0.append(_node_row(plan, i0))
                if plan.is_out[n]:
                    e1.append(64 + int(plan.node_types[n]) - plan.out_start)
                else:
                    e1.append(_node_row(plan, i1))
            e0 += [0] * (G - len(nodes))
            e1 += [0] * (G - len(nodes))
            xlist.extend(e0)
            xlist.extend(e1)
        xidx_cols.append(_wrap_idxs(xlist, st["NX"]))

    widx = np.concatenate(widx_cols, axis=1)
    xidx = np.concatenate(xidx_cols, axis=1)

    initr = np.zeros((128, D), np.float16)
    initr[: plan.num_roots] = root_emb.astype(np.float16)
    initr[64 : 64 + plan.num_out] = output_slot_emb.astype(np.float16)

    return dict(wblob=blob, widx=widx, xidx=xidx, bias1=bias1, bias2=bias2,
                initr=initr)


def _node_row(plan, n):
    n = int(n)
    if n < plan.num_roots:
        return n
    r = int(plan.row_of_node[n])
    assert r >= 0, n
    return r


# --------------------------------------------------------------------------
# Bass program
# --------------------------------------------------------------------------

def _build_nc(plan, D, H, gelu_mode="act"):
    import concourse.bacc as bacc
    import concourse.mybir as mybir
    from concourse import tile

    dt = mybir.dt
    AF = mybir.ActivationFunctionType
    ALU = mybir.AluOpType
    rows_per = 3 * D  # 768

    nc = bacc.Bacc("TRN2", target_bir_lowering=False, debug=False,
                   enable_asserts=False, num_devices=NCORES)

    wblob = nc.dram_tensor("wblob", [5 * rows_per, H], dt.float16,
                           kind="ExternalInput")
    widx = nc.dram_tensor("widx", [128, plan.slots_tot * (rows_per // 16)],
                          dt.int16, kind="ExternalInput")
    xidx_cols = sum(st["NX"] for st in plan.stages) // 16
    xidx = nc.dram_tensor("xidx", [128, xidx_cols], dt.int16,
                          kind="ExternalInput")
    bias1 = nc.dram_tensor("bias1", [128, plan.slots_tot * 8], dt.float32,
                           kind="ExternalInput")
    bias2 = nc.dram_tensor("bias2", [1, plan.slots_tot * D], dt.float16,
                           kind="ExternalInput")
    initr = nc.dram_tensor("initr", [128, D], dt.float16, kind="ExternalInput")
    oout = nc.dram_tensor("oout", [plan.R_tot, D], dt.float32,
                          kind="ExternalOutput")

    RG = [list(range(NCORES))]

    with tile.TileContext(nc) as tc:
        with (
            tc.tile_pool(name="dram", bufs=1, space="DRAM") as dpool,
            tc.tile_pool(name="ccpool", bufs=2, space="DRAM") as ccpool,
            tc.tile_pool(name="cpool", bufs=1) as cpool,
            tc.tile_pool(name="wpool", bufs=3) as wpool,
            tc.tile_pool(name="xpool", bufs=2) as xpool,
            tc.tile_pool(name="hpool", bufs=2) as hpool,
            tc.tile_pool(name="ypool", bufs=3) as ypool,
            tc.tile_pool(name="phpool", bufs=1, space="PSUM") as phpool,
            tc.tile_pool(name="pypool", bufs=3, space="PSUM") as pypool,
        ):
            bufH = dpool.tile([plan.bufH_rows, D], dt.float16, name="bufH")

            widx_sb = cpool.tile(list(widx.shape), dt.int16, name="widx_sb")
            nc.sync.dma_start(widx_sb[:, :], widx.ap())
            xidx_sb = cpool.tile(list(xidx.shape), dt.int16, name="xidx_sb")
            nc.sync.dma_start(xidx_sb[:, :], xidx.ap())
            bias1_sb = cpool.tile(list(bias1.shape), dt.float32, name="bias1_sb")
            nc.sync.dma_start(bias1_sb[:, :], bias1.ap())
            bias2_sb = cpool.tile(list(bias2.shape), dt.float16, name="bias2_sb")
            nc.sync.dma_start(bias2_sb[:, :], bias2.ap())

            init_sb = cpool.tile([128, D], dt.float16, name="init_sb")
            nc.sync.dma_start(init_sb[:, :], initr.ap())
            nc.sync.dma_start(bufH[0:128, :], init_sb[:, :])

            ones_sb = cpool.tile([1, 128], dt.float16, name="ones_sb")
            nc.gpsimd.memset(ones_sb[:, :], 1.0)
            eps_sb = cpool.tile([128, 1], dt.float32, name="eps_sb")
            nc.gpsimd.memset(eps_sb[:, :], 1e-24)

            xoff = 0
            for si, st in enumerate(plan.stages):
                NX, R, Gs = st["NX"], st["R"], st["Gs"]
                pre = st["pre"]
                xt = xpool.tile([128, 2, NX], dt.float16, tag="xt",
                                name=f"xt{si}")
                nc.gpsimd.dma_gather(
                    xt[:, :, :], bufH[:, :],
                    xidx_sb[:, xoff : xoff + NX // 16],
                    NX, NX, D, transpose=True,
                )
                xoff += NX // 16

                ccin = ccpool.tile([plan.Rmax, D], dt.float16, tag="cc",
                                   name=f"cc{si}")

                col = 0
                for k, G in enumerate(Gs):
                    sl = st["slot_ids"][k]
                    Gp = 64
                    while Gp < G:
                        Gp *= 2
                    wt = wpool.tile([128, rows_per // 128, H], dt.float16,
                                    tag="wt", name=f"wt_{si}_{k}")
                    nc.gpsimd.dma_gather(
                        wt[:, :, :], wblob.ap(),
                        widx_sb[:, sl * (rows_per // 16) : (sl + 1) * (rows_per // 16)],
                        rows_per, rows_per, H,
                    )

                    ph = phpool.tile([128, 8 * Gp], dt.float32, tag="ph",
                                     name=f"ph_{si}_{k}")
                    for m in range(8):
                        for kk in range(4):
                            rhs = xt[:, kk % 2,
                                     col + (kk // 2) * G : col + (kk // 2) * G + G]
                            nc.tensor.matmul(
                                ph[:, m * Gp : m * Gp + G],
                                wt[:, kk, m * 128 : (m + 1) * 128],
                                rhs,
                                start=(kk == 0), stop=(kk == 3),
                            )
                    hsb = hpool.tile([128, 8, Gp], dt.float16, tag="h",
                                     name=f"h_{si}_{k}")
                    for m in range(8):
                        pslice = ph[:, m * Gp : m * Gp + G]
                        bslice = bias1_sb[:, sl * 8 + m : sl * 8 + m + 1]
                        if gelu_mode == "act":
                            nc.scalar.activation(hsb[:, m, 0:G], pslice,
                                                 AF.Gelu, bias=bslice)
                        else:  # tanh-approx composition (sim-checkable)
                            xg = ypool.tile([128, GCAP], dt.float32, tag="gx",
                                            name=f"gx_{si}_{k}_{m}")
                            nc.scalar.activation(xg[:, 0:G], pslice,
                                                 AF.Identity, bias=bslice)
                            sg = ypool.tile([128, GCAP], dt.float32, tag="gs",
                                            name=f"gs_{si}_{k}_{m}")
                            nc.scalar.activation(sg[:, 0:G], xg[:, 0:G],
                                                 AF.Square)
                            nc.vector.tensor_scalar(sg[:, 0:G], sg[:, 0:G],
                                                    0.044715, 1.0,
                                                    ALU.mult, ALU.add)
                            nc.vector.tensor_tensor(sg[:, 0:G], sg[:, 0:G],
                                                    xg[:, 0:G], ALU.mult)
                            nc.scalar.activation(sg[:, 0:G], sg[:, 0:G],
                                                 AF.Tanh,
                                                 scale=0.7978845608028654)
                            nc.vector.tensor_scalar(sg[:, 0:G], sg[:, 0:G],
                                                    0.5, 0.5,
                                                    ALU.mult, ALU.add)
                            nc.vector.tensor_tensor(hsb[:, m, 0:G],
                                                    sg[:, 0:G], xg[:, 0:G],
                                                    ALU.mult)
                    for c0 in range(0, G, 128):
                        g = min(128, G - c0)
                        py = pypool.tile([128, D], dt.float32, tag="py",
                                         name=f"py_{si}_{k}_{c0}")
                        for kk in range(8):
                            nc.tensor.matmul(
                                py[0:g, :],
                                hsb[:, kk, c0 : c0 + g],
                                wt[:, 4 + kk // 4,
                                   (kk % 4) * D : (kk % 4 + 1) * D],
                                start=(kk == 0), stop=False,
                            )
                        nc.tensor.matmul(
                            py[0:g, :], ones_sb[0:1, 0:g],
                            bias2_sb[0:1, sl * D : (sl + 1) * D],
                            start=False, stop=True,
                        )
                        sq = ypool.tile([128, D], dt.float16, tag="sq",
                                        name=f"sq_{si}_{k}_{c0}")
                        ss = ypool.tile([128, 1], dt.float32, tag="ss",
                                        name=f"ss_{si}_{k}_{c0}")
                        nc.scalar.activation(sq[0:g, :], py[0:g, :], AF.Square,
                                             accum_out=ss[0:g, :])
                        sr = ypool.tile([128, 1], dt.float32, tag="sr",
                                        name=f"sr_{si}_{k}_{c0}")
                        nc.scalar.activation(sr[0:g, :], ss[0:g, :], AF.Sqrt,
                                             scale=1.0 / D,
                                             bias=eps_sb[0:g, :])
                        inv = ypool.tile([128, 1], dt.float32, tag="inv",
                                         name=f"inv_{si}_{k}_{c0}")
                        nc.vector.reciprocal(inv[0:g, :], sr[0:g, :])
                        y32 = ypool.tile([128, D], dt.float32, tag="y32",
                                         name=f"y32_{si}_{k}_{c0}")
                        nc.vector.tensor_scalar_mul(y32[0:g, :], py[0:g, :],
                                                    inv[0:g, :])
                        y16 = ypool.tile([128, D], dt.float16, tag="y16",
                                         name=f"y16_{si}_{k}_{c0}")
                        nc.vector.tensor_copy(y16[0:g, :], y32[0:g, :])
                        r0 = st["out_off"] + pre[k] + c0
                        nc.sync.dma_start(oout.ap()[r0 : r0 + g, :], y32[0:g, :])
                        nc.sync.dma_start(ccin[pre[k] + c0 : pre[k] + c0 + g, :],
                                          y16[0:g, :])
                    col += 2 * G

                nc.gpsimd.collective_compute(
                    "AllGather",
                    mybir.AluOpType.bypass,
                    replica_groups=RG,
                    ins=[ccin[0:R, :]],
                    outs=[bufH[128 + st["off"] : 128 + st["off"] + NCORES * R, :]],
                )

    nc.compile()
    return nc


# --------------------------------------------------------------------------
# Entry point
# --------------------------------------------------------------------------

_CACHE = {}


def _prepare(node_inputs, node_types, num_roots, num_trunk, num_out):
    key = (node_inputs.tobytes(), node_types.tobytes())
    if key in _CACHE:
        return _CACHE[key]
    plan = _build_plan(node_inputs, node_types, num_roots, num_trunk, num_out)
    _CACHE[key] = plan
    return plan


def kernel(node_inputs_indices, node_types, root_emb, output_slot_emb,
           W1, b1, W2, b2):
    node_inputs_indices = np.asarray(node_inputs_indices)
    node_types = np.asarray(node_types)
    root_emb = np.asarray(root_emb, np.float32)
    output_slot_emb = np.asarray(output_slot_emb, np.float32)
    W1 = np.asarray(W1, np.float32)
    b1 = np.asarray(b1, np.float32)
    W2 = np.asarray(W2, np.float32)
    b2 = np.asarray(b2, np.float32)

    num_trunk = W1.shape[0] - 1
    num_roots = root_emb.shape[0]
    num_out = output_slot_emb.shape[0]
    plan = _prepare(node_inputs_indices, node_types, num_roots, num_trunk,
                    num_out)
    D = root_emb.shape[1]
    H = W1.shape[2]

    nckey = ("nc", node_inputs_indices.tobytes(), node_types.tobytes())
    if nckey in _CACHE:
        nc = _CACHE[nckey]
    else:
        nc = _build_nc(plan, D, H)
        _CACHE[nckey] = nc

    in_maps = [
        _build_core_inputs(plan, c, W1, b1, W2, b2, root_emb, output_slot_emb)
        for c in range(NCORES)
    ]

    import os

    from concourse import bass_utils
    trace = bool(int(os.environ.get("DAG_KERNEL_TRACE", "0")))
    try:
        res = bass_utils.run_bass_kernel_spmd(nc, in_maps, list(range(NCORES)),
                                              trace=trace)
    except ModuleNotFoundError:
        res = bass_utils.run_bass_kernel_spmd(nc, in_maps, list(range(NCORES)),
                                              trace=False)
    global LAST_RESULTS
    LAST_RESULTS = res
    outs = [res.results[c]["oout"] for c in range(NCORES)]

    full = np.zeros((plan.N, D), np.float32)
    full[: plan.num_roots] = root_emb
    for n, (c, r) in plan.outpos_of_node.items():
        full[n] = outs[c][r]
    return full


# revision 35
# speedup vs baseline: 5.6013x; 5.6013x over previous
"""Trainium2 Bass kernel: DagnabbitAutoEncoder sequential DAG sweep.

Strategy (8 NeuronCores, SPMD single program, per-core data):
  - Host computes topological levels and a deadline-forced, eager-filled
    pass schedule: nodes are batched into per-(stage, encoder-type) passes;
    28 stages (= DAG depth), one AllGather per stage to exchange computed
    embeddings (fp16) through a shared DRAM buffer `bufH`.
  - Trunk encoder types are partitioned 4-per-core; the shared output-node
    encoder is replicated. Per pass, the core dma_gathers that type's
    weight blob (W1 + repacked W2, fp16 rows) from its private DRAM stack
    into an SBUF slot buffer — fully prefetchable.
  - Parent embeddings are fetched with dma_gather(transpose=True), which
    lands gathered bufH rows as columns: exactly the X^T [512, G] layout
    the tensor engine needs (matmul contracts over the partition dim).
  - Stage 1 GEMM: W1 tiles stationary, X^T moving -> H^T in PSUM;
    ScalarE applies bias+exact-GELU and casts to fp16.
    Stage 2 GEMM: H^T stationary, W2 tiles moving -> Y [g,256] in PSUM;
    bias2 via a ones-row K=1 matmul; VectorE/ScalarE normalize rows to
    sqrt(D); results stored fp32 (final output) + fp16 (exchange).
  - All cores run an identical instruction stream; per-core differences
    (types, node ids, gather indices, weights) are carried in input data,
    padded so shapes/counts match across cores.
"""

import math
import sys

import numpy as np

if "/opt/trn_rl_repo" not in sys.path:
    sys.path.insert(0, "/opt/trn_rl_repo")

NCORES = 8
GCAP = 256  # max nodes per pass slot
TYPES_PER_CORE = 4


# --------------------------------------------------------------------------
# Host-side schedule
# --------------------------------------------------------------------------

class Plan:
    pass


def _build_plan(node_inputs, node_types, num_roots, num_trunk, num_out):
    N = node_inputs.shape[0]
    out_start = num_trunk + num_roots
    is_out = node_types >= out_start
    enc = np.where(is_out, num_trunk, np.clip(node_types, 0, num_trunk - 1))

    # ASAP levels
    level = np.zeros(N, np.int64)
    ni = node_inputs
    for n in range(num_roots, N):
        i0, i1 = ni[n]
        level[n] = (level[i0] + 1) if is_out[n] else max(level[i0], level[i1]) + 1
    S = int(level.max())

    # ALAP deadlines
    alap = np.full(N, S, np.int64)
    for n in range(N - 1, num_roots - 1, -1):
        i0, i1 = ni[n]
        a = alap[n] - 1
        if alap[i0] > a:
            alap[i0] = a
        if (not is_out[n]) and alap[i1] > a:
            alap[i1] = a

    # Greedy deadline-forced scheduling with per-core eager fill.
    # Core for trunk type t is t // TYPES_PER_CORE; output enc (num_trunk) is
    # replicated so its nodes can go anywhere.
    scheduled = np.zeros(N, bool)
    scheduled[:num_roots] = True
    remaining = list(range(num_roots, N))

    stages = []  # list of per-stage dicts
    for s in range(1, S + 1):
        elig = [
            n
            for n in remaining
            if scheduled[ni[n][0]] and (is_out[n] or scheduled[ni[n][1]])
        ]
        by_enc = {}
        for n in elig:
            by_enc.setdefault(int(enc[n]), []).append(n)
        forced = {
            t for t, nodes in by_enc.items() if any(alap[n] == s for n in nodes)
        }
        if not forced:
            continue

        # per-core pass lists of (local_type_idx, [nodes])
        core_passes = [[] for _ in range(NCORES)]

        def add_type(t):
            nodes = by_enc.pop(t)
            if t == num_trunk:
                return nodes  # handled by caller (splittable)
            c = t // TYPES_PER_CORE
            lti = t % TYPES_PER_CORE
            for a in range(0, len(nodes), GCAP):
                core_passes[c].append((lti, nodes[a : a + GCAP]))
            return None

        out_pool = []
        for t in sorted(forced):
            r = add_type(t)
            if r is not None:
                out_pool = r

        K = max(len(p) for p in core_passes)
        K = max(K, 1)

        # place forced output nodes: cores with slack first
        if out_pool:
            chunks = [out_pool[a : a + GCAP] for a in range(0, len(out_pool), GCAP)]
            for ch in chunks:
                c = min(range(NCORES), key=lambda c: len(core_passes[c]))
                core_passes[c].append((TYPES_PER_CORE, ch))
            K = max(K, max(len(p) for p in core_passes))

        # eager fill: cores with fewer passes than K pick up their own
        # eligible (unforced) types, largest first
        for c in range(NCORES):
            while len(core_passes[c]) < K:
                own = [
                    t
                    for t in by_enc
                    if t != num_trunk and t // TYPES_PER_CORE == c
                ]
                if not own:
                    break
                t = max(own, key=lambda t: len(by_enc[t]))
                lti = t % TYPES_PER_CORE
                nodes = by_enc.pop(t)
                for a in range(0, len(nodes), GCAP):
                    core_passes[c].append((lti, nodes[a : a + GCAP]))
            core_passes[c] = core_passes[c][: max(K, len(core_passes[c]))]
        K = max(len(p) for p in core_passes)

        # sort each core's passes by size desc, pad with dummy passes
        for c in range(NCORES):
            core_passes[c].sort(key=lambda p: -len(p[1]))
            while len(core_passes[c]) < K:
                core_passes[c].append((0, []))

        Gs = [
            max(len(core_passes[c][k][1]) for c in range(NCORES))
            for k in range(K)
        ]
        Gs = [max(g, 1) for g in Gs]

        newly = []
        for c in range(NCORES):
            for _, nodes in core_passes[c]:
                newly.extend(nodes)
        for n in newly:
            scheduled[n] = True
        newset = set(newly)
        remaining = [n for n in remaining if n not in newset]

        stages.append(dict(s=s, core_passes=core_passes, Gs=Gs))

    assert not remaining, f"{len(remaining)} nodes unscheduled"

    # ---- layout: bufH rows, cc offsets, output rows, global slot ids ----
    plan = Plan()
    plan.S = len(stages)
    plan.stages = stages
    slot_id = 0
    bufh_off = 0  # offset after the 128 static rows
    out_off = 0
    stage_of_node = np.full(N, -1, np.int64)
    within_of_node = np.full(N, -1, np.int64)  # offset within the stage's AG out
    outpos_of_node = {}  # node -> (core, oout row)
    for sidx, st in enumerate(stages):
        Gs = st["Gs"]
        K = len(Gs)
        st["slot_ids"] = list(range(slot_id, slot_id + K))
        slot_id += K
        R = sum(Gs)
        st["R"] = R
        st["off"] = bufh_off
        st["out_off"] = out_off
        pre = np.concatenate([[0], np.cumsum(Gs)]).astype(int)
        st["pre"] = pre
        NX = 2 * R
        NX = ((NX + 127) // 128) * 128
        st["NX"] = NX
        for c in range(NCORES):
            for k, (lti, nodes) in enumerate(st["core_passes"][c]):
                for i, n in enumerate(nodes):
                    stage_of_node[n] = sidx
                    within_of_node[n] = c * R + pre[k] + i
                    outpos_of_node[n] = (c, out_off + pre[k] + i)
        bufh_off += NCORES * R
        out_off += R
    plan.bufH_rows = 128 + bufh_off
    plan.R_tot = out_off
    plan.slots_tot = slot_id
    plan.Rmax = max(st["R"] for st in stages)
    plan.NXmax = max(st["NX"] for st in stages)
    plan.Gmax = max(max(st["Gs"]) for st in stages)
    assert plan.bufH_rows < 32768, plan.bufH_rows
    # default ("local" xmode) row offsets: AG_s output starts at 128 + off_s
    plan.stage_rowoff = [128 + st["off"] for st in stages]
    plan.stage_of_node = stage_of_node
    plan.within_of_node = within_of_node
    plan.outpos_of_node = outpos_of_node
    plan.enc = enc
    plan.is_out = is_out
    plan.N = N
    plan.num_roots = num_roots
    plan.num_trunk = num_trunk
    plan.num_out = num_out
    plan.out_start = out_start
    plan.node_inputs = node_inputs
    plan.node_types = node_types
    return plan


def _wrap_idxs(idx_list, num_idxs):
    """int16 index layout for dma_gather: [128, num_idxs//16], index i at
    partition i%16, column i//16, replicated across the 8 Q7 16-partition
    groups."""
    a = np.zeros(num_idxs, np.int16)
    a[: len(idx_list)] = np.asarray(idx_list, np.int16)
    a = a.reshape(num_idxs // 16, 16).T  # [16, cols]
    return np.tile(a, (8, 1))  # [128, cols]


def _build_core_inputs(plan, core, W1, b1, W2, b2, root_emb, output_slot_emb):
    """Per-core input arrays (shapes identical across cores)."""
    num_trunk = plan.num_trunk
    D = root_emb.shape[1]
    H = W1.shape[2]
    assert D == 256 and H == 1024 and W1.shape[1] == 2 * D
    ni_types = [core * TYPES_PER_CORE + j for j in range(TYPES_PER_CORE)] + [num_trunk]

    # weight blob: per local type, 768 rows of 1024 fp16
    # rows 0..511   = W1[t]  (512 x 1024)
    # rows 512..767 = repacked W2[t]: blob[512 + q*128 + p] =
    #                 concat_j W2[t][(4q+j)*128 + p, :]  (j = 0..3)
    rows_per = 3 * D  # 768
    blob = np.zeros((5 * rows_per, H), np.float16)
    for li, t in enumerate(ni_types):
        w1 = W1[t].astype(np.float16)  # [512, 1024]
        blob[li * rows_per : li * rows_per + 2 * D] = w1
        w2 = W2[t].astype(np.float16).reshape(2, 4, 128, D)
        w2 = w2.transpose(0, 2, 1, 3).reshape(2 * 128, 4 * D)  # [256, 1024]
        blob[li * rows_per + 2 * D : (li + 1) * rows_per] = w2

    # per-slot tables
    widx_cols = []
    xidx_cols = []
    bias1 = np.zeros((128, plan.slots_tot * 8), np.float32)
    bias2 = np.zeros((1, plan.slots_tot * D), np.float16)
    nH = H // 128  # number of 128-row b1 tiles (8)
    for st in plan.stages:
        xlist = []
        for k, (lti, nodes) in enumerate(st["core_passes"][core]):
            G = st["Gs"][k]
            sl = st["slot_ids"][k]
            widx_cols.append(_wrap_idxs(lti * rows_per + np.arange(rows_per), rows_per))
            t = ni_types[lti]
            bias1[:, sl * nH : (sl + 1) * nH] = (
                b1[t].astype(np.float32).reshape(nH, 128).T
            )
            bias2[0, sl * D : (sl + 1) * D] = b2[t].astype(np.float16)
            e0 = []
            e1 = []
            for n in nodes:
                i0, i1 = plan.node_inputs[n]
                e0.append(_node_row(plan, i0))
                if plan.is_out[n]:
                    e1.append(64 + int(plan.node_types[n]) - plan.out_start)
                else:
                    e1.append(_node_row(plan, i1))
            e0 += [0] * (G - len(nodes))
            e1 += [0] * (G - len(nodes))
            xlist.extend(e0)
            xlist.extend(e1)
        xidx_cols.append(_wrap_idxs(xlist, st["NX"]))

    widx = np.concatenate(widx_cols, axis=1)
    xidx = np.concatenate(xidx_cols, axis=1)

    initr = np.zeros((128, D), np.float16)
    initr[: plan.num_roots] = root_emb.astype(np.float16)
    initr[64 : 64 + plan.num_out] = output_slot_emb.astype(np.float16)

    return dict(wblob=blob, widx=widx, xidx=xidx, bias1=bias1, bias2=bias2,
                initr=initr)


def _node_row(plan, n):
    n = int(n)
    if n < plan.num_roots:
        return n
    s = int(plan.stage_of_node[n])
    assert s >= 0, n
    return plan.stage_rowoff[s] + int(plan.within_of_node[n])


# --------------------------------------------------------------------------
# Bass program
# --------------------------------------------------------------------------

def _build_nc(plan, D, H, gelu_mode="act", repeat=1, xmode="shared",
              skip_ag=False, norm_mode="rsqrt", cc_on_sync=False):
    import concourse.bacc as bacc
    import concourse.mybir as mybir
    from concourse import tile
    from concourse.bass import _add_dep_helper

    dt = mybir.dt
    AF = mybir.ActivationFunctionType
    ALU = mybir.AluOpType
    rows_per = 3 * D  # 768

    nc = bacc.Bacc("TRN2", target_bir_lowering=False, debug=False,
                   enable_asserts=False, num_devices=NCORES)

    wblob = nc.dram_tensor("wblob", [5 * rows_per, H], dt.float16,
                           kind="ExternalInput")
    widx = nc.dram_tensor("widx", [128, plan.slots_tot * (rows_per // 16)],
                          dt.int16, kind="ExternalInput")
    xidx_cols = sum(st["NX"] for st in plan.stages) // 16
    xidx = nc.dram_tensor("xidx", [128, xidx_cols], dt.int16,
                          kind="ExternalInput")
    bias1 = nc.dram_tensor("bias1", [128, plan.slots_tot * 8], dt.float32,
                           kind="ExternalInput")
    bias2 = nc.dram_tensor("bias2", [1, plan.slots_tot * D], dt.float16,
                           kind="ExternalInput")
    initr = nc.dram_tensor("initr", [128, D], dt.float16, kind="ExternalInput")
    oout = nc.dram_tensor("oout", [plan.R_tot, D], dt.float32,
                          kind="ExternalOutput")

    RG = [list(range(NCORES))]

    with tile.TileContext(nc) as tc:
        with (
            tc.tile_pool(name="dram", bufs=1, space="DRAM") as dpool,
            tc.tile_pool(name="ccpool", bufs=2, space="DRAM") as ccpool,
            tc.tile_pool(name="cpool", bufs=1) as cpool,
            tc.tile_pool(name="wpool", bufs=5) as wpool,
            tc.tile_pool(name="xpool", bufs=2) as xpool,
            tc.tile_pool(name="hpool", bufs=2) as hpool,
            tc.tile_pool(name="ypool", bufs=3) as ypool,
            tc.tile_pool(name="phpool", bufs=1, space="PSUM") as phpool,
            tc.tile_pool(name="pypool", bufs=3, space="PSUM") as pypool,
        ):
            if xmode == "local":
                bufH = dpool.tile([plan.bufH_rows, D], dt.float16,
                                  name="bufH")
                gbase = bufH
            else:
                statics = dpool.tile([128, D], dt.float16,
                                     addr_space="Shared", name="statics")
                gbase = statics
            ago_tiles = []  # rep-0 per-stage AG output tiles (shared mode)

            widx_sb = cpool.tile(list(widx.shape), dt.int16, name="widx_sb")
            nc.sync.dma_start(widx_sb[:, :], widx.ap())
            xidx_sb = cpool.tile(list(xidx.shape), dt.int16, name="xidx_sb")
            nc.sync.dma_start(xidx_sb[:, :], xidx.ap())
            bias1_sb = cpool.tile(list(bias1.shape), dt.float32, name="bias1_sb")
            nc.sync.dma_start(bias1_sb[:, :], bias1.ap())
            bias2_sb = cpool.tile(list(bias2.shape), dt.float16, name="bias2_sb")
            nc.sync.dma_start(bias2_sb[:, :], bias2.ap())

            init_sb = cpool.tile([128, D], dt.float16, name="init_sb")
            nc.sync.dma_start(init_sb[:, :], initr.ap())
            nc.sync.dma_start(gbase[0:128, :], init_sb[:, :])

            ones_sb = cpool.tile([1, 128], dt.float16, name="ones_sb")
            nc.gpsimd.memset(ones_sb[:, :], 1.0)
            eps_sb = cpool.tile([128, 1], dt.float32, name="eps_sb")
            nc.gpsimd.memset(eps_sb[:, :], 1e-24)

            prev_cc = None
            for rep in range(repeat):
              xoff = 0
              for si0, st in enumerate(plan.stages):
                si = f"{rep}_{si0}"
                NX, R, Gs = st["NX"], st["R"], st["Gs"]
                pre = st["pre"]
                xt = xpool.tile([128, 2, NX], dt.float16, tag="xt",
                                name=f"xt{si}")
                g_inst = nc.gpsimd.dma_gather(
                    xt[:, :, :], gbase[:, :],
                    xidx_sb[:, xoff : xoff + NX // 16],
                    NX, NX, D, transpose=True,
                )
                if xmode != "local" and prev_cc is not None:
                    _add_dep_helper(g_inst.ins, prev_cc.ins, True,
                                    "gather reads prior AG outputs")
                xoff += NX // 16

                ccin = ccpool.tile([plan.Rmax, D], dt.float16, tag="cc",
                                   name=f"cc{si}")

                col = 0
                for k, G in enumerate(Gs):
                    sl = st["slot_ids"][k]
                    Gp = 64
                    while Gp < G:
                        Gp *= 2
                    wt = wpool.tile([128, rows_per // 128, H], dt.float16,
                                    tag="wt", name=f"wt_{si}_{k}")
                    nc.gpsimd.dma_gather(
                        wt[:, :, :], wblob.ap(),
                        widx_sb[:, sl * (rows_per // 16) : (sl + 1) * (rows_per // 16)],
                        rows_per, rows_per, H,
                    )

                    ph = phpool.tile([128, 8 * Gp], dt.float32, tag="ph",
                                     name=f"ph_{si}_{k}")
                    for m in range(8):
                        for kk in range(4):
                            rhs = xt[:, kk % 2,
                                     col + (kk // 2) * G : col + (kk // 2) * G + G]
                            nc.tensor.matmul(
                                ph[:, m * Gp : m * Gp + G],
                                wt[:, kk, m * 128 : (m + 1) * 128],
                                rhs,
                                start=(kk == 0), stop=(kk == 3),
                            )
                    hsb = hpool.tile([128, 8, Gp], dt.float16, tag="h",
                                     name=f"h_{si}_{k}")
                    for m in range(8):
                        pslice = ph[:, m * Gp : m * Gp + G]
                        bslice = bias1_sb[:, sl * 8 + m : sl * 8 + m + 1]
                        if gelu_mode == "act":
                            nc.scalar.activation(hsb[:, m, 0:G], pslice,
                                                 AF.Gelu, bias=bslice)
                        else:  # tanh-approx composition (sim-checkable)
                            xg = ypool.tile([128, GCAP], dt.float32, tag="gx",
                                            name=f"gx_{si}_{k}_{m}")
                            nc.scalar.activation(xg[:, 0:G], pslice,
                                                 AF.Identity, bias=bslice)
                            sg = ypool.tile([128, GCAP], dt.float32, tag="gs",
                                            name=f"gs_{si}_{k}_{m}")
                            nc.scalar.activation(sg[:, 0:G], xg[:, 0:G],
                                                 AF.Square)
                            nc.vector.tensor_scalar(sg[:, 0:G], sg[:, 0:G],
                                                    0.044715, 1.0,
                                                    ALU.mult, ALU.add)
                            nc.vector.tensor_tensor(sg[:, 0:G], sg[:, 0:G],
                                                    xg[:, 0:G], ALU.mult)
                            nc.scalar.activation(sg[:, 0:G], sg[:, 0:G],
                                                 AF.Tanh,
                                                 scale=0.7978845608028654)
                            nc.vector.tensor_scalar(sg[:, 0:G], sg[:, 0:G],
                                                    0.5, 0.5,
                                                    ALU.mult, ALU.add)
                            nc.vector.tensor_tensor(hsb[:, m, 0:G],
                                                    sg[:, 0:G], xg[:, 0:G],
                                                    ALU.mult)
                    for c0 in range(0, G, 128):
                        g = min(128, G - c0)
                        py = pypool.tile([128, D], dt.float32, tag="py",
                                         name=f"py_{si}_{k}_{c0}")
                        for kk in range(8):
                            nc.tensor.matmul(
                                py[0:g, :],
                                hsb[:, kk, c0 : c0 + g],
                                wt[:, 4 + kk // 4,
                                   (kk % 4) * D : (kk % 4 + 1) * D],
                                start=(kk == 0), stop=False,
                            )
                        nc.tensor.matmul(
                            py[0:g, :], ones_sb[0:1, 0:g],
                            bias2_sb[0:1, sl * D : (sl + 1) * D],
                            start=False, stop=True,
                        )
                        sq = ypool.tile([128, D], dt.float16, tag="sq",
                                        name=f"sq_{si}_{k}_{c0}")
                        ss = ypool.tile([128, 1], dt.float32, tag="ss",
                                        name=f"ss_{si}_{k}_{c0}")
                        nc.scalar.activation(sq[0:g, :], py[0:g, :], AF.Square,
                                             accum_out=ss[0:g, :])
                        inv = ypool.tile([128, 1], dt.float32, tag="inv",
                                         name=f"inv_{si}_{k}_{c0}")
                        if norm_mode == "rsqrt":
                            nc.scalar.activation(inv[0:g, :], ss[0:g, :],
                                                 AF.Abs_reciprocal_sqrt,
                                                 scale=1.0 / D,
                                                 bias=eps_sb[0:g, :])
                        else:
                            sr = ypool.tile([128, 1], dt.float32, tag="sr",
                                            name=f"sr_{si}_{k}_{c0}")
                            nc.scalar.activation(sr[0:g, :], ss[0:g, :],
                                                 AF.Sqrt, scale=1.0 / D,
                                                 bias=eps_sb[0:g, :])
                            nc.vector.reciprocal(inv[0:g, :], sr[0:g, :])
                        y16 = ypool.tile([128, D], dt.float16, tag="y16",
                                         name=f"y16_{si}_{k}_{c0}")
                        nc.vector.tensor_scalar_mul(y16[0:g, :], py[0:g, :],
                                                    inv[0:g, :])
                        nc.sync.dma_start(ccin[pre[k] + c0 : pre[k] + c0 + g, :],
                                          y16[0:g, :])
                        y32 = ypool.tile([128, D], dt.float32, tag="y32",
                                         name=f"y32_{si}_{k}_{c0}")
                        nc.vector.tensor_scalar_mul(y32[0:g, :], py[0:g, :],
                                                    inv[0:g, :])
                        r0 = st["out_off"] + pre[k] + c0
                        nc.sync.dma_start(oout.ap()[r0 : r0 + g, :], y32[0:g, :])
                    col += 2 * G

                if si0 == len(plan.stages) - 1:
                    # nothing reads the last stage's outputs — no exchange
                    if rep == 0 and xmode != "local":
                        ago_tiles.append(None)
                    continue
                if xmode == "local":
                    ag_out = bufH[128 + st["off"] : 128 + st["off"]
                                  + NCORES * R, :]
                else:
                    ago = dpool.tile([NCORES * R, D], dt.float16,
                                     addr_space="Shared", bufs=1,
                                     name=f"ago{si}")
                    if rep == 0:
                        ago_tiles.append(ago)
                    ag_out = ago[:, :]
                if skip_ag:
                    # timing-decomposition mode: serialize stages via a tiny
                    # DRAM copy instead of the collective
                    prev_cc = nc.sync.dma_start(ag_out[0:R, :], ccin[0:R, :])
                else:
                    cc_eng = nc.sync if cc_on_sync else nc.gpsimd
                    prev_cc = cc_eng.collective_compute(
                        "AllGather",
                        mybir.AluOpType.bypass,
                        replica_groups=RG,
                        ins=[ccin[0:R, :]],
                        outs=[ag_out],
                    )

    nc.compile()

    if xmode != "local":
        base_addr = nc.lookup_mls(gbase.tensor).memorylocations[0].addr
        rowbytes = D * 2
        rowoff = []
        for sidx, ago in enumerate(ago_tiles):
            if ago is None:
                rowoff.append(0)  # last stage: never referenced by gathers
                continue
            a = nc.lookup_mls(ago.tensor).memorylocations[0].addr
            off = a - base_addr
            assert off % rowbytes == 0, (sidx, off)
            r = off // rowbytes
            assert 0 < r and r + ago.shape[0] < 32768, (sidx, r)
            rowoff.append(int(r))
        plan.stage_rowoff = rowoff
    else:
        plan.stage_rowoff = [128 + st["off"] for st in plan.stages]
    return nc


# --------------------------------------------------------------------------
# Entry point
# --------------------------------------------------------------------------

_CACHE = {}


def _prepare(node_inputs, node_types, num_roots, num_trunk, num_out):
    key = (node_inputs.tobytes(), node_types.tobytes())
    if key in _CACHE:
        return _CACHE[key]
    plan = _build_plan(node_inputs, node_types, num_roots, num_trunk, num_out)
    _CACHE[key] = plan
    return plan


def kernel(node_inputs_indices, node_types, root_emb, output_slot_emb,
           W1, b1, W2, b2):
    node_inputs_indices = np.asarray(node_inputs_indices)
    node_types = np.asarray(node_types)
    root_emb = np.asarray(root_emb, np.float32)
    output_slot_emb = np.asarray(output_slot_emb, np.float32)
    W1 = np.asarray(W1, np.float32)
    b1 = np.asarray(b1, np.float32)
    W2 = np.asarray(W2, np.float32)
    b2 = np.asarray(b2, np.float32)

    num_trunk = W1.shape[0] - 1
    num_roots = root_emb.shape[0]
    num_out = output_slot_emb.shape[0]
    plan = _prepare(node_inputs_indices, node_types, num_roots, num_trunk,
                    num_out)
    D = root_emb.shape[1]
    H = W1.shape[2]

    nckey = ("nc", node_inputs_indices.tobytes(), node_types.tobytes())
    if nckey in _CACHE:
        nc = _CACHE[nckey]
    else:
        nc = _build_nc(plan, D, H)
        _CACHE[nckey] = nc

    in_maps = [
        _build_core_inputs(plan, c, W1, b1, W2, b2, root_emb, output_slot_emb)
        for c in range(NCORES)
    ]

    import os

    from concourse import bass_utils
    trace = bool(int(os.environ.get("DAG_KERNEL_TRACE", "0")))
    try:
        res = bass_utils.run_bass_kernel_spmd(nc, in_maps, list(range(NCORES)),
                                              trace=trace)
    except ModuleNotFoundError:
        res = bass_utils.run_bass_kernel_spmd(nc, in_maps, list(range(NCORES)),
                                              trace=False)
    global LAST_RESULTS
    LAST_RESULTS = res
    outs = [res.results[c]["oout"] for c in range(NCORES)]

    full = np.zeros((plan.N, D), np.float32)
    full[: plan.num_roots] = root_emb
    for n, (c, r) in plan.outpos_of_node.items():
        full[n] = outs[c][r]
    return full
